# revision 2
# baseline (speedup 1.0000x reference)
"""Bass/Trainium2 kernel for nn_Channel_attention (bottom-16 channel gather).

reference semantics (per sample b):
    weight = mean(x[b], axis=(H, W))           # [C]
    idx    = argsort(weight)[:16]              # ascending pooled value
    out[b] = x[b, idx]                         # [16, H, W]

Strategy: pure data parallel, B=16 sharded 2 samples per core over 8 cores.
Per core (x shard viewed as [512, 16384] = [(sample, channel), H*W]):
  1. Stream 32x [128, 2048] tiles, DVE reduce_add -> per-channel partial sums.
  2. PE-transpose sums into per-sample rows [2, 256]; negate; two rounds of
     max8/max_index/match_replace -> bottom-16 channel indices in ascending
     order of pooled sum (argsort of sum == argsort of mean).
  3. Expand the 16 indices to 128 row-indices (idx*8 + subrow) with two tiny
     PE matmuls, then SWDGE indirect-gather [128, 2048] per sample and store
     contiguously to the output.
"""

import sys

if "/opt/trn_rl_repo" not in sys.path:
    sys.path.insert(0, "/opt/trn_rl_repo")

import numpy as np

from concourse import bacc, mybir, tile
from concourse.bass import IndirectOffsetOnAxis
from concourse.bass_utils import run_bass_kernel_spmd
from concourse.masks import make_identity

N_CORES = 8
B, C, H, W = 16, 256, 128, 128
K = 16
BPC = B // N_CORES          # samples per core = 2
E = H * W                   # 16384 elems per channel
CH = 2048                   # chunk width (1 MiB tiles)
NJ = E // CH                # 8 chunks per (sample, channel-half)
ROWS = BPC * C              # 512 channel rows per core
GROWS = ROWS * (E // CH)    # 4096 gather rows of CH elems

f32 = mybir.dt.float32
i32 = mybir.dt.int32
u32 = mybir.dt.uint32
X = mybir.AxisListType.X
Alu = mybir.AluOpType

_cache = {}


def _build():
    nc = bacc.Bacc("TRN2", target_bir_lowering=False, debug=False,
                   num_devices=N_CORES)
    x_d = nc.dram_tensor("x", [ROWS, E], f32, kind="ExternalInput")
    y_d = nc.dram_tensor("y", [BPC * K * NJ, CH], f32, kind="ExternalOutput")

    with tile.TileContext(nc) as tc:
        with (
            tc.tile_pool(name="load", bufs=16) as load_pool,
            tc.tile_pool(name="small", bufs=1) as small,
            tc.tile_pool(name="gather", bufs=2) as gather_pool,
            tc.tile_pool(name="psum", bufs=1, space="PSUM") as psum,
        ):
            ident = small.tile([128, 128], f32)
            make_identity(nc, ident[:])

            # ---- pass 1: per-channel sums ----
            partials = small.tile([128, 4 * NJ], f32)
            for s in range(BPC):
                for h in range(2):
                    base = s * C + h * 128
                    col0 = (h * BPC + s) * NJ
                    for j in range(NJ):
                        t = load_pool.tile([128, CH], f32)
                        nc.sync.dma_start(
                            out=t[:], in_=x_d[base:base + 128,
                                              j * CH:(j + 1) * CH])
                        nc.vector.reduce_sum(
                            out=partials[:, col0 + j:col0 + j + 1],
                            in_=t[:], axis=X)

            sums_col = small.tile([128, 4], f32)
            for c in range(4):
                nc.vector.reduce_sum(out=sums_col[:, c:c + 1],
                                     in_=partials[:, c * NJ:(c + 1) * NJ],
                                     axis=X)

            # ---- sums -> per-sample rows [2, 256], negated ----
            psum_w = psum.tile([BPC, C], f32)
            nc.tensor.matmul(out=psum_w[:, 0:128], lhsT=sums_col[:, 0:BPC],
                             rhs=ident[:], start=True, stop=True)
            nc.tensor.matmul(out=psum_w[:, 128:256], lhsT=sums_col[:, BPC:4],
                             rhs=ident[:], start=True, stop=True)
            w_neg = small.tile([BPC, C], f32)
            nc.scalar.mul(w_neg[:], psum_w[:], -1.0)

            # ---- bottom-16 by two rounds of max8 on -sums ----
            m1 = small.tile([BPC, 8], f32)
            m2 = small.tile([BPC, 8], f32)
            idx_u = small.tile([BPC, K], u32)
            w_rep = small.tile([BPC, C], f32)
            nc.vector.max(out=m1[:], in_=w_neg[:])
            nc.vector.max_index(out=idx_u[:, 0:8], in_max=m1[:],
                                in_values=w_neg[:])
            nc.vector.match_replace(out=w_rep[:], in_to_replace=m1[:],
                                    in_values=w_neg[:], imm_value=-1e38)
            nc.vector.max(out=m2[:], in_=w_rep[:])
            nc.vector.max_index(out=idx_u[:, 8:16], in_max=m2[:],
                                in_values=w_rep[:])
            idx_f = small.tile([BPC, K], f32)
            nc.vector.tensor_copy(idx_f[:], idx_u[:])

            # ---- expand to 128 gather-row indices per sample ----
            # idx128[p, s] = (s*C + idx[s, p>>3]) * 8 + (p & 7)
            psum_t = psum.tile([K, BPC], f32)
            nc.tensor.transpose(out=psum_t[:], in_=idx_f[:],
                                identity=ident[0:BPC, 0:BPC])
            idx_t = small.tile([K, BPC], f32)
            nc.vector.tensor_copy(idx_t[:], psum_t[:])

            e_i = small.tile([K, 128], i32)
            nc.gpsimd.iota(out=e_i[:], pattern=[[1, 128]], base=0,
                           channel_multiplier=0)
            nc.vector.tensor_scalar(out=e_i[:], in0=e_i[:], scalar1=3,
                                    scalar2=None, op0=Alu.arith_shift_right)
            e_f = small.tile([K, 128], f32)
            nc.vector.tensor_copy(e_f[:], e_i[:])
            col_i = small.tile([K, 1], i32)
            nc.gpsimd.iota(out=col_i[:], pattern=[[1, 1]], base=0,
                           channel_multiplier=1)
            col_f = small.tile([K, 1], f32)
            nc.vector.tensor_copy(col_f[:], col_i[:])
            e_mat = small.tile([K, 128], f32)
            nc.vector.tensor_scalar(out=e_mat[:], in0=e_f[:], scalar1=col_f[:],
                                    scalar2=None, op0=Alu.is_equal)

            psum_e = psum.tile([128, BPC], f32)
            nc.tensor.matmul(out=psum_e[:], lhsT=e_mat[:], rhs=idx_t[:],
                             start=True, stop=True)

            pp = small.tile([128, 1], i32)
            nc.gpsimd.iota(out=pp[:], pattern=[[1, 1]], base=0,
                           channel_multiplier=1)
            nc.vector.tensor_scalar(out=pp[:], in0=pp[:], scalar1=7,
                                    scalar2=None, op0=Alu.bitwise_and)
            a7f = small.tile([128, 1], f32)
            nc.vector.tensor_copy(a7f[:], pp[:])
            s_off = small.tile([128, BPC], f32)
            for s in range(BPC):
                nc.vector.memset(s_off[:, s:s + 1], float(s * C * NJ))

            idx128_f = small.tile([128, BPC], f32)
            nc.vector.tensor_scalar(out=idx128_f[:], in0=psum_e[:],
                                    scalar1=float(NJ), scalar2=None,
                                    op0=Alu.mult)
            nc.vector.tensor_tensor(out=idx128_f[:], in0=idx128_f[:],
                                    in1=a7f[:].to_broadcast([128, BPC]),
                                    op=Alu.add)
            nc.vector.tensor_tensor(out=idx128_f[:], in0=idx128_f[:],
                                    in1=s_off[:], op=Alu.add)
            idx128_i = small.tile([128, BPC], i32)
            nc.vector.tensor_copy(idx128_i[:], idx128_f[:])

            # ---- gather the selected channels, store contiguously ----
            xg = x_d[:].rearrange("r (u e) -> (r u) e", u=NJ)
            for s in range(BPC):
                g = gather_pool.tile([128, CH], f32)
                nc.gpsimd.indirect_dma_start(
                    out=g[:], out_offset=None, in_=xg,
                    in_offset=IndirectOffsetOnAxis(
                        ap=idx128_i[:, s:s + 1], axis=0))
                nc.sync.dma_start(out=y_d[s * 128:(s + 1) * 128, :], in_=g[:])

    nc.compile()
    return nc


def get_nc():
    if "nc" not in _cache:
        _cache["nc"] = _build()
    return _cache["nc"]


def make_in_maps(x: np.ndarray) -> list[dict[str, np.ndarray]]:
    x = np.ascontiguousarray(np.asarray(x, dtype=np.float32))
    assert x.shape == (B, C, H, W)
    return [{"x": x[c * BPC:(c + 1) * BPC].reshape(ROWS, E)}
            for c in range(N_CORES)]


def assemble(results: list[dict[str, np.ndarray]]) -> np.ndarray:
    out = np.empty((B, K, H, W), dtype=np.float32)
    for c in range(N_CORES):
        out[c * BPC:(c + 1) * BPC] = results[c]["y"].reshape(BPC, K, H, W)
    return out


def kernel(x: np.ndarray) -> np.ndarray:
    nc = get_nc()
    res = run_bass_kernel_spmd(nc, make_in_maps(x), list(range(N_CORES)))
    return assemble(res.results)
